# revision 4
# baseline (speedup 1.0000x reference)
"""Chamfer distance L2 (B=4, N=M=8192, D=3) on 8 TRN2 NeuronCores — banded
with an exactness certificate + host fix-up.

Sharding: core c handles batch b = c//2, xyz1-half h = c%2.

Banded retrieval: host sorts both clouds of a batch by the z coordinate.
Query tile t (128 sorted xyz1 points) is compared only against xyz2
candidates whose sorted rank lies in a W-wide window around the tile's
own rank: c0(t) = clamp(128t+64-W/2, 0, M-W). Each core computes a
[4096 x W] strip of the [4096 x 8192] distance matrix (W=512: 1/16 of
the work on every engine vs the full all-pairs kernel).

Exactness: for a query at sorted rank r with banded min m, every
excluded candidate differs in z by at least the window-edge gap g, so
d >= g^2. If m <= g^2*CERT the banded min is PROVABLY the global min
(CERT covers the bf16 rounding of d). The host checks this certificate
per point (it has the sorted z arrays) and recomputes the handful of
failures (outlier points, ~200 of 65536) exactly in numpy — O(flags*M)
host work, zero device work. Correct for ANY input distribution.

For h=1 cores the host REVERSES the sorted order of both clouds so the
clamped window offsets are identical compile-time constants on every
core (single SPMD program).

Device per 4-tile quad: K=18 augmented split-bf16 matmuls (near-f32
accurate d at full bf16 PE rate) -> 4 PSUM banks; one ScalarE copy to
SBUF bf16; VectorE min-accumulates the column stream (dist2, one TT per
tile) and runs a batched 3D row-min pyramid (dist1, two TTs per quad +
one segmented reduce at the end). dist2 tail: DMA-xbar block-transpose
of the covered 34 column blocks (started mid-loop as blocks finalize) +
3D TT pyramid + one segmented reduce. GpSimd seeds the column
accumulator. Host: certificate + fix-up + means.
"""

import sys

for _p in ("/opt/trn_rl_repo",):
    if _p not in sys.path:
        sys.path.insert(0, _p)

from contextlib import ExitStack

import numpy as np
import ml_dtypes

import concourse.bacc as bacc
import concourse.bass as bass
import concourse.mybir as mybir
import concourse.tile as tile
from concourse.bass_utils import run_bass_kernel_spmd

WEIGHT = 0.6
B = 4
N = 8192
M = 8192
D = 3
NCORES = 8
HALF = N // 2  # xyz1 rows per core = 4096

P = 128
NT = HALF // P  # 32 n-tiles per core
W = 256  # band width (xyz2 candidates per n-tile)
K = 18
QUAD = 8  # n-tiles per PSUM group
BIG = 3.0e38
CERT = 0.98  # certificate safety factor (bf16 rounding of d)

F32 = mybir.dt.float32
BF16 = mybir.dt.bfloat16
MIN = mybir.AluOpType.min
AX = mybir.AxisListType.X
BF = ml_dtypes.bfloat16

_cached = {}


def _c0(t):
    return min(max(t * P + P // 2 - W // 2, 0), M - W)


# covered column blocks per core: [0, c0(NT-1)+W) -> NB blocks of 128
NB = (_c0(NT - 1) + W + P - 1) // P


def _build():
    nc = bacc.Bacc(
        "TRN2",
        target_bir_lowering=False,
        debug=False,
        enable_asserts=False,
        num_devices=NCORES,
    )

    lhs_d = nc.dram_tensor("lhs", [K, HALF], BF16, kind="ExternalInput")
    rhs_d = nc.dram_tensor("rhs", [K, NB * P], BF16, kind="ExternalInput")
    out1_d = nc.dram_tensor("out1", [P, NT], F32, kind="ExternalOutput")
    out2_d = nc.dram_tensor("out2", [P, NB * P], BF16, kind="ExternalOutput")

    GW = QUAD * W  # free size of one PSUM group
    W2, W4 = W // 2, W // 4

    with tile.TileContext(nc) as tc, ExitStack() as ctx:
        const = ctx.enter_context(tc.tile_pool(name="const", bufs=1))
        ckpool = ctx.enter_context(tc.tile_pool(name="ck", bufs=4))
        rapool = ctx.enter_context(tc.tile_pool(name="ra", bufs=4))
        psum = ctx.enter_context(tc.tile_pool(name="ps", bufs=2, space="PSUM"))

        lhs_sb = const.tile([K, HALF], BF16)
        rhs_sb = const.tile([K, NB * P], BF16)
        colacc = const.tile([P, NB * P], BF16)
        rowbuf = const.tile([P, NT * W4], BF16)
        dist1 = const.tile([P, NT], F32)

        RH = NB * P // 2
        nc.sync.dma_start(rhs_sb[:, 0:RH], rhs_d[:, 0:RH])
        nc.sync.dma_start(lhs_sb[:, 0:1024], lhs_d[:, 0:1024])
        nc.sync.dma_start(rhs_sb[:, RH:], rhs_d[:, RH:])
        nc.sync.dma_start(lhs_sb[:, 1024:], lhs_d[:, 1024:])

        covered_end = _c0(NT - 1) + W
        if covered_end < NB * P:
            nc.vector.memset(colacc[:, covered_end : NB * P], BIG)

        def colacc_out(lo_c, hi_c):
            # finalized column range -> DRAM; host does the 128-way min
            nc.sync.dma_start(out2_d[:, lo_c:hi_c], colacc[:, lo_c:hi_c])

        prev_cks = None
        for q in range(NT // QUAD):
            pt = psum.tile([P, GW], F32, tag="ps")
            ck = ckpool.tile([P, GW], BF16, tag="ck")
            for i in range(QUAD):
                nt = QUAD * q + i
                c0 = _c0(nt)
                nc.tensor.matmul(
                    pt[:, i * W : (i + 1) * W],
                    lhs_sb[:, nt * P : (nt + 1) * P],
                    rhs_sb[:, c0 : c0 + W],
                    start=True,
                    stop=True,
                )
            cks = ck[:].rearrange("p (i x) -> p i x", x=W)
            # column-min: every column is covered by exactly two tiles
            # (window stride 128 = W/2), and each region is first-touch:
            # the batched pairwise mins WRITE colacc directly; oct seams
            # are a single pairwise TT reading both octs' ck tiles.
            if q == 0:
                # split the first copy so VectorE starts ~2us earlier
                nc.scalar.copy(ck[:, 0 : GW // 2], pt[:, 0 : GW // 2])
                nc.scalar.copy(ck[:, GW // 2 : GW], pt[:, GW // 2 : GW])
                # tile 0/1 clamped windows: cols [0,64) only tile 0;
                # [64,192) tiles 0+1; [192,256) tiles 0+1+2 (uu pair (1,2)
                # writes [192,320), then fold in e0's top quarter).
                nc.vector.tensor_copy(colacc[:, 0:64], cks[:, 0, 0:64])
                nc.vector.tensor_tensor(
                    colacc[:, 64:192], cks[:, 0, 64:192], cks[:, 1, 0:W2], MIN
                )
                uA = colacc[:, 192 : 192 + 2 * P].rearrange(
                    "p (i x) -> p i x", x=P
                )
                nc.vector.tensor_tensor(
                    uA, cks[:, 1:3, W2:W], cks[:, 2:4, 0:W2], MIN
                )
                nc.vector.tensor_tensor(
                    colacc[:, 192:256], colacc[:, 192:256], cks[:, 0, 192:W], MIN
                )
                uB = colacc[:, 192 + 2 * P : 192 + 6 * P].rearrange(
                    "p (i x) -> p i x", x=P
                )
                nc.vector.tensor_tensor(
                    uB, cks[:, 3:7, W2:W], cks[:, 4:8, 0:W2], MIN
                )
            else:
                nc.scalar.copy(ck[:], pt[:])
                lo = _c0(QUAD * q)
                nseg = QUAD - 1
                uus = colacc[:, lo + W2 : lo + W2 + nseg * P].rearrange(
                    "p (i x) -> p i x", x=P
                )
                nc.vector.tensor_tensor(
                    uus,
                    cks[:, 0:nseg, W2:W],
                    cks[:, 1 : 1 + nseg, 0:W2],
                    MIN,
                )
                # seam with the previous oct: also first-touch
                nc.vector.tensor_tensor(
                    colacc[:, lo : lo + W2],
                    prev_cks[:, QUAD - 1, W2:W],
                    cks[:, 0, 0:W2],
                    MIN,
                )
            if q == NT // QUAD - 1:
                hi = _c0(QUAD * q + QUAD - 1)
                nc.vector.tensor_copy(
                    colacc[:, hi + W2 : hi + W], cks[:, QUAD - 1, W2:W]
                )
            prev_cks = cks
            # ship column ranges no longer touched by later tiles
            if q == 1:
                colacc_out(0, 1792)
            elif q == 2:
                colacc_out(1792, 3008)
            elif q == 3:
                colacc_out(3008, NB * P)
            # row-min pyramid: two batched 3D TT levels per oct + reduce per pair
            t1 = rapool.tile([P, QUAD * W2], BF16, tag="t1")
            t1s = t1[:].rearrange("p (i x) -> p i x", x=W2)
            nc.vector.tensor_tensor(t1s, cks[:, :, 0:W2], cks[:, :, W2:W], MIN)
            rb = rowbuf[:, q * QUAD * W4 : (q + 1) * QUAD * W4].rearrange(
                "p (i x) -> p i x", x=W4
            )
            nc.vector.tensor_tensor(rb, t1s[:, :, 0:W4], t1s[:, :, W4:W2], MIN)
            if q == 1:
                nc.vector.tensor_reduce(
                    dist1[:, 0 : 2 * QUAD],
                    rowbuf[:, 0 : 2 * QUAD * W4].rearrange(
                        "p (i x) -> p i x", x=W4
                    ),
                    axis=AX,
                    op=MIN,
                )

        nc.vector.tensor_reduce(
            dist1[:, 2 * QUAD : 4 * QUAD],
            rowbuf[:, 2 * QUAD * W4 : 4 * QUAD * W4].rearrange(
                "p (i x) -> p i x", x=W4
            ),
            axis=AX,
            op=MIN,
        )
        nc.sync.dma_start(out1_d[:], dist1[:])

    nc.compile()
    return nc


def _get_nc():
    if "nc" not in _cached:
        _cached["nc"] = _build()
    return _cached["nc"]


def _split3(v):
    h = v.astype(BF)
    r = v - h.astype(np.float64)
    m = r.astype(BF)
    l = (r - m.astype(np.float64)).astype(BF)
    return h, m, l


def _prep(xyz1, xyz2):
    xs = []
    ys = []
    for b in range(B):
        xs.append(xyz1[b][np.argsort(xyz1[b][:, 2], kind="stable")])
        ys.append(xyz2[b][np.argsort(xyz2[b][:, 2], kind="stable")])
    return xs, ys


def _in_maps(xs, ys):
    maps = []
    for c in range(NCORES):
        b, h = divmod(c, 2)
        X = xs[b][h * HALF : (h + 1) * HALF].astype(np.float64)
        Y = ys[b].astype(np.float64)
        if h == 1:
            X = X[::-1]
            Y = Y[::-1]

        xh = X.astype(BF)
        xl = (X - xh.astype(np.float64)).astype(BF)
        yh = Y.astype(BF)
        yl = (Y - yh.astype(np.float64)).astype(BF)
        Xr = xh.astype(np.float64) + xl.astype(np.float64)
        Yr = yh.astype(np.float64) + yl.astype(np.float64)
        s1h, s1m, s1l = _split3(np.einsum("nd,nd->n", Xr, Xr))
        s2h, s2m, s2l = _split3(np.einsum("md,md->m", Yr, Yr))

        lhs = np.empty((K, HALF), BF)
        lhs[0:3] = 1.0
        lhs[3] = s1h
        lhs[4] = s1m
        lhs[5] = s1l
        lhs[6:9] = (-2.0 * xh.astype(np.float64)).astype(BF).T
        lhs[9:12] = lhs[6:9]
        lhs[12:15] = (-2.0 * xl.astype(np.float64)).astype(BF).T
        lhs[15:18] = lhs[12:15]

        rhs = np.empty((K, M), BF)
        rhs[0] = s2h
        rhs[1] = s2m
        rhs[2] = s2l
        rhs[3:6] = 1.0
        rhs[6:9] = yh.T
        rhs[9:12] = yl.T
        rhs[12:15] = yh.T
        rhs[15:18] = yl.T
        maps.append({"lhs": lhs, "rhs": np.ascontiguousarray(rhs[:, : NB * P])})
    return maps


def _combine(results, xs, ys):
    c0s = np.array([_c0(t) for t in range(NT)])
    d1_all = []
    d2_all = []
    for b in range(B):
        zx = xs[b][:, 2].astype(np.float64)
        zy = ys[b][:, 2].astype(np.float64)
        X = xs[b].astype(np.float64)
        Y = ys[b].astype(np.float64)

        # ---- dist1: device values + per-row window bounds ----
        d1 = np.empty(N)
        lo1 = np.empty(N, dtype=int)  # window [lo, hi) in global y ranks
        hi1 = np.empty(N, dtype=int)
        for h in range(2):
            v = results[2 * b + h]["out1"].T.reshape(-1)  # local rank r = t*128+p
            if h == 0:
                d1[0:HALF] = v
                lo1[0:HALF] = np.repeat(c0s, P)
                hi1[0:HALF] = np.repeat(c0s + W, P)
            else:
                gr = N - 1 - np.arange(HALF)  # local r -> global rank
                d1[gr] = v
                lo1[gr] = M - np.repeat(c0s + W, P)
                hi1[gr] = M - np.repeat(c0s, P)
        gl = np.where(lo1 > 0, np.maximum(zx - zy[np.maximum(lo1 - 1, 0)], 0.0), np.inf)
        gr_ = np.where(
            hi1 < M, np.maximum(zy[np.minimum(hi1, M - 1)] - zx, 0.0), np.inf
        )
        g = np.minimum(gl, gr_)
        bad = np.where(d1 > g * g * CERT)[0]
        for o in range(0, len(bad), 512):
            ix = bad[o : o + 512]
            d1[ix] = ((X[ix][:, None, :] - Y[None, :, :]) ** 2).sum(-1).min(1)
        d1_all.append(d1)

        # ---- dist2: host 128-way min over the raw column accumulator ----
        a = results[2 * b]["out2"].astype(np.float32).min(axis=0).astype(np.float64)
        z = results[2 * b + 1]["out2"].astype(np.float32).min(axis=0).astype(np.float64)
        d2 = np.full(M, np.inf)
        d2[0 : NB * P] = a
        d2[M - NB * P : M] = np.minimum(d2[M - NB * P : M], z[::-1])
        cov_lo = np.full(M, N, dtype=int)  # covered x-rank interval per column
        cov_hi = np.full(M, -1, dtype=int)
        for h in range(2):
            for t in range(NT):
                c0 = c0s[t]
                if h == 0:
                    cs, ce, rs, re = c0, c0 + W, t * P, (t + 1) * P
                else:
                    cs, ce = M - (c0 + W), M - c0
                    rs, re = N - (t + 1) * P, N - t * P
                cov_lo[cs:ce] = np.minimum(cov_lo[cs:ce], rs)
                cov_hi[cs:ce] = np.maximum(cov_hi[cs:ce], re - 1)
        gl = np.where(
            cov_lo > 0, np.maximum(zy - zx[np.maximum(cov_lo - 1, 0)], 0.0), np.inf
        )
        gr_ = np.where(
            cov_hi < N - 1,
            np.maximum(zx[np.minimum(cov_hi + 1, N - 1)] - zy, 0.0),
            np.inf,
        )
        g = np.minimum(gl, gr_)
        with np.errstate(invalid="ignore"):
            bad = np.where(~(d2 <= g * g * CERT) | (cov_hi < 0))[0]
        for o in range(0, len(bad), 512):
            ix = bad[o : o + 512]
            d2[ix] = ((Y[ix][:, None, :] - X[None, :, :]) ** 2).sum(-1).min(1)
        d2_all.append(d2)

    d1 = np.concatenate(d1_all)
    d2 = np.concatenate(d2_all)
    val = WEIGHT * (np.float64(d1.mean()) + np.float64(d2.mean())) / 2.0
    return np.float32(val)


def run(xyz1, xyz2, trace=False, **spmd_kwargs):
    xyz1 = np.ascontiguousarray(np.asarray(xyz1, dtype=np.float32))
    xyz2 = np.ascontiguousarray(np.asarray(xyz2, dtype=np.float32))
    xs, ys = _prep(xyz1, xyz2)
    nc = _get_nc()
    br = run_bass_kernel_spmd(
        nc, _in_maps(xs, ys), list(range(NCORES)), trace=trace, **spmd_kwargs
    )
    return _combine(br.results, xs, ys), br


def kernel(xyz1, xyz2):
    out, _ = run(xyz1, xyz2)
    return out


if __name__ == "__main__":
    rng = np.random.default_rng(0)
    a = rng.standard_normal((B, N, D)).astype(np.float32)
    b = rng.standard_normal((B, M, D)).astype(np.float32)
    print(kernel(a, b))
